# revision 2
# baseline (speedup 1.0000x reference)
"""MoE-routed dynamics ensemble kernel for 8 Trainium2 NeuronCores.

Reference computes all 7 expert MLPs densely for every sample and then
gathers one expert per sample (idx in [0, TOP_K)).  Here we route instead:
sort samples by expert on the host, spread every expert's samples evenly
across the 8 cores, and run only the routed expert per sample as dense
per-expert matmuls in a feature-major layout ([features, samples]), which
needs no on-device transposes.  The Gaussian-sampling epilogue
(clip / exp / mu + std * eps, next_state = state + delta) runs on-chip.
"""

import sys

if "/opt/trn_rl_repo" not in sys.path:
    sys.path.insert(0, "/opt/trn_rl_repo")

import numpy as np

import concourse.bass as bass
import concourse.mybir as mybir
import concourse.tile as tile
import bass_rust
from concourse.bass_utils import run_bass_kernel_spmd

N_CORES = 8
HIDDEN = 512
P = 128
NT = 512          # samples per matmul moving-operand tile
F32 = mybir.dt.float32
BF16 = mybir.dt.bfloat16

# "f32" = exact-ish, "bf16" = bf16 matmuls (f32 epilogue)
MODE = "f32"


def _split_multi_waits(nc):
    """This walrus build supports one semaphore wait per instruction; hoist
    extra waits onto NoOps placed just before the over-subscribed one."""
    counter = 0
    for f in nc.m.functions:
        for bb in f.blocks:
            new = []
            changed = False
            for inst in bb.instructions:
                si = inst.sync_info
                if si is not None and len(si.on_wait) > 1:
                    waits = list(si.on_wait)
                    for w in waits[:-1]:
                        counter += 1
                        nop = mybir.InstNoOp(
                            name=f"waitsplit-{counter}", ins=[], outs=[]
                        )
                        nop.engine = inst.engine
                        nop.sync_info = bass_rust.SyncInfo(
                            on_wait=[w], on_update=[]
                        )
                        new.append(nop)
                    inst.sync_info = bass_rust.SyncInfo(
                        on_wait=[waits[-1]], on_update=list(si.on_update)
                    )
                    changed = True
                new.append(inst)
            if changed:
                bb.instructions = new


def _build(slots, n_col, in_dim, state_dim, out_half, mode):
    """Build the SPMD Bass program.

    slots: list of capacities (columns) per expert slot, one slot per used
    expert, identical on every core.  n_col = sum(slots).
    """
    n_e = len(slots)
    kt = HIDDEN // P                     # contraction tiles for layers 2/3
    mt = HIDDEN // P                     # output row tiles for layers 1/2
    f32 = F32
    mdt = f32 if mode == "f32" else BF16  # matmul operand dtype
    relu = mybir.ActivationFunctionType.Relu
    expf = mybir.ActivationFunctionType.Exp
    alu = mybir.AluOpType

    nc = bass.Bass("TRN2", debug=False)
    xt_d = nc.dram_tensor("xt", [in_dim, n_col], f32, kind="ExternalInput")
    ep_d = nc.dram_tensor("epst", [out_half, n_col], f32, kind="ExternalInput")
    w1_d = nc.dram_tensor("w1", [in_dim, n_e, HIDDEN], mdt, kind="ExternalInput")
    w2_d = nc.dram_tensor("w2", [P, n_e * kt, HIDDEN], mdt, kind="ExternalInput")
    w3_d = nc.dram_tensor("w3", [P, n_e * kt, 2 * out_half], mdt, kind="ExternalInput")
    b1_d = nc.dram_tensor("b1", [P, n_e * mt], f32, kind="ExternalInput")
    b2_d = nc.dram_tensor("b2", [P, n_e * mt], f32, kind="ExternalInput")
    b3_d = nc.dram_tensor("b3", [out_half, n_e, 2], f32, kind="ExternalInput")
    yt_d = nc.dram_tensor("yt", [out_half, n_col], f32, kind="ExternalOutput")

    with tile.TileContext(nc) as tc:
        with (
            tc.tile_pool(name="singles", bufs=1) as singles,
            tc.tile_pool(name="psum", bufs=8, space="PSUM") as psum,
            tc.tile_pool(name="acts", bufs=6) as acts,
            tc.tile_pool(name="epi", bufs=3) as epi,
        ):
            xt_s = singles.tile([in_dim, n_col], f32)
            nc.sync.dma_start(out=xt_s[:], in_=xt_d[:])
            if mode != "f32":
                # SWDGE casts during DMA; re-read x as bf16 for the matmuls
                xb_s = singles.tile([in_dim, n_col], mdt, tag="xb")
                nc.gpsimd.dma_start(out=xb_s[:], in_=xt_d[:])
            else:
                xb_s = xt_s
            ep_s = singles.tile([out_half, n_col], f32, tag="ep")
            nc.sync.dma_start(out=ep_s[:], in_=ep_d[:])
            w1_s = singles.tile([in_dim, n_e, HIDDEN], mdt, tag="w1")
            nc.sync.dma_start(out=w1_s[:], in_=w1_d[:])
            w3_s = singles.tile([P, n_e * kt, 2 * out_half], mdt, tag="w3")
            nc.sync.dma_start(out=w3_s[:], in_=w3_d[:])
            b1_s = singles.tile([P, n_e * mt], f32, tag="b1")
            nc.sync.dma_start(out=b1_s[:], in_=b1_d[:])
            b2_s = singles.tile([P, n_e * mt], f32, tag="b2")
            nc.sync.dma_start(out=b2_s[:], in_=b2_d[:])
            b3_s = singles.tile([out_half, n_e, 2], f32, tag="b3")
            nc.sync.dma_start(out=b3_s[:], in_=b3_d[:])
            w2_s = singles.tile([P, n_e * kt, HIDDEN], mdt, tag="w2")
            for s in range(n_e):
                # split per expert so later experts' weights stream in
                # behind the first experts' compute
                nc.sync.dma_start(
                    out=w2_s[:, s * kt : (s + 1) * kt, :],
                    in_=w2_d[:, s * kt : (s + 1) * kt, :],
                )

            off = 0
            for s, cap in enumerate(slots):
                for t0 in range(0, cap, NT):
                    n = min(NT, cap - t0)
                    c0 = off + t0
                    xs = xb_s[:, c0 : c0 + n]
                    # ---- layer 1: [in_dim -> HIDDEN] ----
                    a1 = []
                    for m in range(mt):
                        ps = psum.tile([P, n], f32, tag="ps")
                        nc.tensor.matmul(
                            ps,
                            w1_s[:, s, m * P : (m + 1) * P],
                            xs,
                            start=True,
                            stop=True,
                        )
                        a = acts.tile([P, n], mdt, tag="a1")
                        j = s * mt + m
                        nc.scalar.activation(a, ps, relu, bias=b1_s[:, j : j + 1])
                        a1.append(a)
                    # ---- layer 2: [HIDDEN -> HIDDEN] ----
                    a2 = []
                    for m in range(mt):
                        ps = psum.tile([P, n], f32, tag="ps")
                        for k in range(kt):
                            nc.tensor.matmul(
                                ps,
                                w2_s[:, s * kt + k, m * P : (m + 1) * P],
                                a1[k],
                                start=(k == 0),
                                stop=(k == kt - 1),
                            )
                        a = acts.tile([P, n], mdt, tag="a2")
                        j = s * mt + m
                        if m % 2 == 0:
                            nc.scalar.activation(a, ps, relu, bias=b2_s[:, j : j + 1])
                        else:
                            nc.vector.tensor_scalar(
                                a, ps, b2_s[:, j : j + 1], 0.0, op0=alu.add, op1=alu.max
                            )
                        a2.append(a)
                    # ---- layer 3: [HIDDEN -> 2*out_half], split mu | log_std ----
                    ps_mu = psum.tile([out_half, n], f32, tag="ps")
                    for k in range(kt):
                        nc.tensor.matmul(
                            ps_mu,
                            w3_s[:, s * kt + k, 0:out_half],
                            a2[k],
                            start=(k == 0),
                            stop=(k == kt - 1),
                        )
                    ps_ls = psum.tile([out_half, n], f32, tag="ps")
                    for k in range(kt):
                        nc.tensor.matmul(
                            ps_ls,
                            w3_s[:, s * kt + k, out_half : 2 * out_half],
                            a2[k],
                            start=(k == 0),
                            stop=(k == kt - 1),
                        )
                    # ---- epilogue: y = mu + exp(clip(ls, -20, 2)) * eps ----
                    t_ls = epi.tile([out_half, n], f32, tag="ls")
                    nc.vector.tensor_scalar(
                        t_ls, ps_ls, b3_s[:, s, 1:2], 2.0, op0=alu.add, op1=alu.min
                    )
                    nc.vector.tensor_scalar_max(t_ls, t_ls, -20.0)
                    t_std = epi.tile([out_half, n], f32, tag="std")
                    nc.scalar.activation(t_std, t_ls, expf)
                    t_y = epi.tile([out_half, n], f32, tag="y")
                    nc.vector.tensor_mul(t_y, t_std, ep_s[:, c0 : c0 + n])
                    t_mu = epi.tile([out_half, n], f32, tag="mu")
                    nc.vector.tensor_scalar_add(t_mu, ps_mu, b3_s[:, s, 0:1])
                    nc.vector.tensor_add(t_y, t_y, t_mu)
                    nc.vector.tensor_add(
                        t_y[0:state_dim],
                        t_y[0:state_dim],
                        xt_s[0:state_dim, c0 : c0 + n],
                    )
                    nc.sync.dma_start(out=yt_d[:, c0 : c0 + n], in_=t_y)
                off += cap

    _split_multi_waits(nc)
    return nc


_CACHE = {}


def _get_nc(key, *args):
    if key not in _CACHE:
        _CACHE[key] = _build(*args)
    return _CACHE[key]


def run(inputs, trace=False):
    state = np.asarray(inputs["state"], dtype=np.float32)
    action = np.asarray(inputs["action"], dtype=np.float32)
    eps = np.asarray(inputs["eps"], dtype=np.float32)
    idx = np.asarray(inputs["idx"]).astype(np.int64)
    W1 = np.asarray(inputs["W1"], dtype=np.float32)
    b1 = np.asarray(inputs["b1"], dtype=np.float32)
    W2 = np.asarray(inputs["W2"], dtype=np.float32)
    b2 = np.asarray(inputs["b2"], dtype=np.float32)
    W3 = np.asarray(inputs["W3"], dtype=np.float32)
    b3 = np.asarray(inputs["b3"], dtype=np.float32)

    B, state_dim = state.shape
    in_dim = state_dim + action.shape[1]
    out_half = state_dim + 1
    n_ens = W1.shape[0]

    x = np.concatenate([state, action], axis=1)  # [B, in_dim]

    # ---- host routing: group samples by expert, balance across cores ----
    counts = np.bincount(idx, minlength=n_ens)
    experts = [e for e in range(n_ens) if counts[e] > 0]
    order = np.argsort(idx, kind="stable")
    seg_off = np.concatenate([[0], np.cumsum(counts)])

    slots = []
    for e in experts:
        cap = -(-int(counts[e]) // N_CORES)       # ceil
        cap = -(-cap // 4) * 4                     # mult of 4 cols (16B)
        slots.append(cap)
    n_col = sum(slots)

    # gather index per (core, column); -1 = padding
    gidx = np.full((N_CORES, n_col), -1, dtype=np.int64)
    off = 0
    for si, e in enumerate(experts):
        seg = order[seg_off[e] : seg_off[e + 1]]
        n = len(seg)
        base, rem = divmod(n, N_CORES)
        p = 0
        for c in range(N_CORES):
            ln = base + (1 if c < rem else 0)
            gidx[c, off : off + ln] = seg[p : p + ln]
            p += ln
        off += slots[si]

    valid = gidx >= 0
    gsafe = np.where(valid, gidx, 0)

    # ---- per-core inputs (feature-major) ----
    mode = MODE
    mnp = np.float32 if mode == "f32" else None
    if mode != "f32":
        import ml_dtypes

        mnp = ml_dtypes.bfloat16

    in_maps = []
    kt = HIDDEN // P
    w1p = np.ascontiguousarray(W1[experts].transpose(1, 0, 2)).astype(mnp)
    w2p = np.ascontiguousarray(
        W2[experts].reshape(len(experts), kt, P, HIDDEN)
        .transpose(2, 0, 1, 3)
        .reshape(P, len(experts) * kt, HIDDEN)
    ).astype(mnp)
    w3p = np.ascontiguousarray(
        W3[experts].reshape(len(experts), kt, P, 2 * out_half)
        .transpose(2, 0, 1, 3)
        .reshape(P, len(experts) * kt, 2 * out_half)
    ).astype(mnp)
    b1p = np.ascontiguousarray(
        b1[experts].reshape(len(experts), kt, P).transpose(2, 0, 1).reshape(P, -1)
    )
    b2p = np.ascontiguousarray(
        b2[experts].reshape(len(experts), kt, P).transpose(2, 0, 1).reshape(P, -1)
    )
    b3p = np.ascontiguousarray(
        b3[experts].reshape(len(experts), 2, out_half).transpose(2, 0, 1)
    )

    for c in range(N_CORES):
        xc = x[gsafe[c]]
        xc[~valid[c]] = 0.0
        ec = eps[gsafe[c]]
        ec[~valid[c]] = 0.0
        in_maps.append(
            {
                "xt": np.ascontiguousarray(xc.T),
                "epst": np.ascontiguousarray(ec.T),
                "w1": w1p,
                "w2": w2p,
                "w3": w3p,
                "b1": b1p,
                "b2": b2p,
                "b3": b3p,
            }
        )

    key = (tuple(slots), n_col, in_dim, state_dim, out_half, mode)
    nc = _get_nc(key, tuple(slots), n_col, in_dim, state_dim, out_half, mode)

    res = run_bass_kernel_spmd(nc, in_maps, list(range(N_CORES)), trace=trace)

    next_state = np.empty((B, state_dim), dtype=np.float32)
    reward = np.empty((B, 1), dtype=np.float32)
    for c in range(N_CORES):
        yt = res.results[c]["yt"]  # [out_half, n_col]
        cols = gidx[c][valid[c]]
        yv = yt[:, valid[c]]
        next_state[cols] = yv[:state_dim].T
        reward[cols, 0] = yv[state_dim]
    return (next_state, reward), res


def kernel(**inputs):
    out, _ = run(inputs)
    return out


# revision 3
# speedup vs baseline: 2.5993x; 2.5993x over previous
"""MoE-routed dynamics ensemble kernel for 8 Trainium2 NeuronCores.

Reference computes all 7 expert MLPs densely for every sample and then
gathers one expert per sample (idx in [0, TOP_K)).  Here we route instead:
sort samples by expert on the host, spread every expert's samples evenly
across the 8 cores, and run only the routed expert per sample as dense
per-expert matmuls in a feature-major layout ([features, samples]), which
needs no on-device transposes.  The Gaussian-sampling epilogue
(clip / exp / mu + std * eps, next_state = state + delta) runs on-chip.
"""

import sys

if "/opt/trn_rl_repo" not in sys.path:
    sys.path.insert(0, "/opt/trn_rl_repo")

import numpy as np

import concourse.bass as bass
import concourse.mybir as mybir
import concourse.tile as tile
import bass_rust
from concourse.bass_utils import run_bass_kernel_spmd

N_CORES = 8
HIDDEN = 512
P = 128
NT = 512          # max free dim per matmul (one PSUM bank of f32)
CB = 1024         # column block: two matmul tiles sharing one weight load
F32 = mybir.dt.float32
BF16 = mybir.dt.bfloat16

# "f32" = exact-ish, "bf16" = bf16 matmuls (f32 accumulate + f32 epilogue)
MODE = "bf16"


def _split_multi_waits(nc):
    """This walrus build supports one semaphore wait per instruction; hoist
    extra waits onto NoOps placed just before the over-subscribed one."""
    counter = 0
    for f in nc.m.functions:
        for bb in f.blocks:
            new = []
            changed = False
            for inst in bb.instructions:
                si = inst.sync_info
                if si is not None and len(si.on_wait) > 1:
                    waits = list(si.on_wait)
                    for w in waits[:-1]:
                        counter += 1
                        nop = mybir.InstNoOp(
                            name=f"waitsplit-{counter}", ins=[], outs=[]
                        )
                        nop.engine = inst.engine
                        nop.sync_info = bass_rust.SyncInfo(
                            on_wait=[w], on_update=[]
                        )
                        new.append(nop)
                    inst.sync_info = bass_rust.SyncInfo(
                        on_wait=[waits[-1]], on_update=list(si.on_update)
                    )
                    changed = True
                new.append(inst)
            if changed:
                bb.instructions = new


def _build(slots, n_col, in_dim, state_dim, out_half, mode):
    """Build the SPMD Bass program.

    slots: list of capacities (columns) per expert slot, one slot per used
    expert, identical on every core.  n_col = sum(slots).
    """
    n_e = len(slots)
    kt = HIDDEN // P                     # contraction tiles for layers 2/3
    mt = HIDDEN // P                     # output row tiles for layers 1/2
    f32 = F32
    mdt = f32 if mode == "f32" else BF16  # matmul operand dtype
    relu = mybir.ActivationFunctionType.Relu
    expf = mybir.ActivationFunctionType.Exp
    ident = mybir.ActivationFunctionType.Identity
    alu = mybir.AluOpType

    nc = bass.Bass("TRN2", debug=False)
    xb_d = nc.dram_tensor("xb", [in_dim, n_col], mdt, kind="ExternalInput")
    st_d = nc.dram_tensor("st", [state_dim, n_col], f32, kind="ExternalInput")
    ep_d = nc.dram_tensor("epst", [out_half, n_col], f32, kind="ExternalInput")
    w1_d = nc.dram_tensor("w1", [in_dim, n_e, HIDDEN], mdt, kind="ExternalInput")
    w2_d = nc.dram_tensor("w2", [P, n_e * kt, HIDDEN], mdt, kind="ExternalInput")
    w3_d = nc.dram_tensor("w3", [P, n_e * kt, 2 * out_half], mdt, kind="ExternalInput")
    b1_d = nc.dram_tensor("b1", [P, n_e * mt], f32, kind="ExternalInput")
    b2_d = nc.dram_tensor("b2", [P, n_e * mt], f32, kind="ExternalInput")
    b3_d = nc.dram_tensor("b3", [out_half, n_e, 2], f32, kind="ExternalInput")
    yt_d = nc.dram_tensor("yt", [out_half, n_col], f32, kind="ExternalOutput")

    with tile.TileContext(nc) as tc:
        with (
            tc.tile_pool(name="singles", bufs=1) as singles,
            tc.tile_pool(name="psum", bufs=4, space="PSUM") as psum,
            tc.tile_pool(name="acts", bufs=6) as acts,
            tc.tile_pool(name="epi", bufs=2) as epi,
        ):
            # DMA issue order = priority order: first expert's operands first
            w1_s = singles.tile([in_dim, n_e, HIDDEN], mdt, tag="w1")
            nc.sync.dma_start(out=w1_s[:], in_=w1_d[:])
            xb_s = singles.tile([in_dim, n_col], mdt, tag="xb")
            nc.sync.dma_start(out=xb_s[:], in_=xb_d[:])
            b1_s = singles.tile([P, n_e * mt], f32, tag="b1")
            nc.sync.dma_start(out=b1_s[:], in_=b1_d[:])
            b2_s = singles.tile([P, n_e * mt], f32, tag="b2")
            nc.sync.dma_start(out=b2_s[:], in_=b2_d[:])
            b3_s = singles.tile([out_half, n_e, 2], f32, tag="b3")
            nc.sync.dma_start(out=b3_s[:], in_=b3_d[:])
            w2_s = singles.tile([P, n_e * kt, HIDDEN], mdt, tag="w2")
            for s in range(n_e):
                # split per expert so later experts' weights stream in
                # behind the first experts' compute
                nc.sync.dma_start(
                    out=w2_s[:, s * kt : (s + 1) * kt, :],
                    in_=w2_d[:, s * kt : (s + 1) * kt, :],
                )
            w3_s = singles.tile([P, n_e * kt, 2 * out_half], mdt, tag="w3")
            nc.sync.dma_start(out=w3_s[:], in_=w3_d[:])
            ep_s = singles.tile([out_half, n_col], f32, tag="ep")
            nc.sync.dma_start(out=ep_s[:], in_=ep_d[:])
            st_s = singles.tile([state_dim, n_col], f32, tag="st")
            nc.sync.dma_start(out=st_s[:], in_=st_d[:])

            off = 0
            for s, cap in enumerate(slots):
                for cb0 in range(0, cap, CB):
                    cb = min(CB, cap - cb0)
                    c0 = off + cb0
                    # sub-tiles within the column block (share weight loads)
                    subs = [(o, min(NT, cb - o)) for o in range(0, cb, NT)]
                    # ---- layer 1: [in_dim -> HIDDEN] ----
                    a1 = []
                    for m in range(mt):
                        ps = psum.tile([P, cb], f32, tag="ps")
                        for o, n in subs:
                            nc.tensor.matmul(
                                ps[:, o : o + n],
                                w1_s[:, s, m * P : (m + 1) * P],
                                xb_s[:, c0 + o : c0 + o + n],
                                start=True,
                                stop=True,
                            )
                        a = acts.tile([P, cb], mdt, tag="a1")
                        j = s * mt + m
                        nc.scalar.activation(a, ps, relu, bias=b1_s[:, j : j + 1])
                        a1.append(a)
                    # ---- layer 2: [HIDDEN -> HIDDEN] ----
                    a2 = []
                    for m in range(mt):
                        ps = psum.tile([P, cb], f32, tag="ps")
                        for k in range(kt):
                            for o, n in subs:
                                nc.tensor.matmul(
                                    ps[:, o : o + n],
                                    w2_s[:, s * kt + k, m * P : (m + 1) * P],
                                    a1[k][:, o : o + n],
                                    start=(k == 0),
                                    stop=(k == kt - 1),
                                )
                        a = acts.tile([P, cb], mdt, tag="a2")
                        j = s * mt + m
                        nc.vector.tensor_scalar(
                            a, ps, b2_s[:, j : j + 1], 0.0, op0=alu.add, op1=alu.max
                        )
                        a2.append(a)
                    # ---- layer 3: [HIDDEN -> 2*out_half], split mu | log_std ----
                    ps_mu = psum.tile([out_half, cb], f32, tag="ps")
                    for k in range(kt):
                        for o, n in subs:
                            nc.tensor.matmul(
                                ps_mu[:, o : o + n],
                                w3_s[:, s * kt + k, 0:out_half],
                                a2[k][:, o : o + n],
                                start=(k == 0),
                                stop=(k == kt - 1),
                            )
                    ps_ls = psum.tile([out_half, cb], f32, tag="ps")
                    for k in range(kt):
                        for o, n in subs:
                            nc.tensor.matmul(
                                ps_ls[:, o : o + n],
                                w3_s[:, s * kt + k, out_half : 2 * out_half],
                                a2[k][:, o : o + n],
                                start=(k == 0),
                                stop=(k == kt - 1),
                            )
                    # ---- epilogue: y = mu + exp(min(ls, 2)) * eps ----
                    # (the reference also clips at -20; below that exp() is
                    # ~2e-9 so skipping the lower clip is far inside fp32
                    # noise for O(1) outputs)
                    t_ls = epi.tile([out_half, cb], f32, tag="ls")
                    nc.vector.tensor_scalar(
                        t_ls, ps_ls, b3_s[:, s, 1:2], 2.0, op0=alu.add, op1=alu.min
                    )
                    t_std = epi.tile([out_half, cb], f32, tag="std")
                    nc.scalar.activation(t_std, t_ls, expf)
                    t_mu = epi.tile([out_half, cb], f32, tag="mu")
                    nc.scalar.activation(t_mu, ps_mu, ident, bias=b3_s[:, s, 0:1])
                    t_y = epi.tile([out_half, cb], f32, tag="y")
                    nc.vector.tensor_mul(t_y, t_std, ep_s[:, c0 : c0 + cb])
                    nc.vector.tensor_add(t_y, t_y, t_mu)
                    nc.gpsimd.tensor_add(
                        t_y[0:state_dim],
                        t_y[0:state_dim],
                        st_s[:, c0 : c0 + cb],
                    )
                    nc.sync.dma_start(out=yt_d[:, c0 : c0 + cb], in_=t_y)
                off += cap

    _split_multi_waits(nc)
    return nc


_CACHE = {}


def _get_nc(key, *args):
    if key not in _CACHE:
        _CACHE[key] = _build(*args)
    return _CACHE[key]


def run(inputs, trace=False):
    state = np.asarray(inputs["state"], dtype=np.float32)
    action = np.asarray(inputs["action"], dtype=np.float32)
    eps = np.asarray(inputs["eps"], dtype=np.float32)
    idx = np.asarray(inputs["idx"]).astype(np.int64)
    W1 = np.asarray(inputs["W1"], dtype=np.float32)
    b1 = np.asarray(inputs["b1"], dtype=np.float32)
    W2 = np.asarray(inputs["W2"], dtype=np.float32)
    b2 = np.asarray(inputs["b2"], dtype=np.float32)
    W3 = np.asarray(inputs["W3"], dtype=np.float32)
    b3 = np.asarray(inputs["b3"], dtype=np.float32)

    B, state_dim = state.shape
    in_dim = state_dim + action.shape[1]
    out_half = state_dim + 1
    n_ens = W1.shape[0]

    x = np.concatenate([state, action], axis=1)  # [B, in_dim]

    # ---- host routing: group samples by expert, balance across cores ----
    counts = np.bincount(idx, minlength=n_ens)
    experts = [e for e in range(n_ens) if counts[e] > 0]
    order = np.argsort(idx, kind="stable")
    seg_off = np.concatenate([[0], np.cumsum(counts)])

    slots = []
    for e in experts:
        cap = -(-int(counts[e]) // N_CORES)       # ceil
        cap = -(-cap // 4) * 4                     # mult of 4 cols (16B)
        slots.append(cap)
    n_col = sum(slots)

    # gather index per (core, column); -1 = padding
    gidx = np.full((N_CORES, n_col), -1, dtype=np.int64)
    off = 0
    for si, e in enumerate(experts):
        seg = order[seg_off[e] : seg_off[e + 1]]
        n = len(seg)
        base, rem = divmod(n, N_CORES)
        p = 0
        for c in range(N_CORES):
            ln = base + (1 if c < rem else 0)
            gidx[c, off : off + ln] = seg[p : p + ln]
            p += ln
        off += slots[si]

    valid = gidx >= 0
    gsafe = np.where(valid, gidx, 0)

    # ---- per-core inputs (feature-major) ----
    mode = MODE
    if mode == "f32":
        mnp = np.float32
    else:
        import ml_dtypes

        mnp = ml_dtypes.bfloat16

    kt = HIDDEN // P
    ne = len(experts)
    w1p = np.ascontiguousarray(W1[experts].transpose(1, 0, 2)).astype(mnp)
    w2p = np.ascontiguousarray(
        W2[experts].reshape(ne, kt, P, HIDDEN)
        .transpose(2, 0, 1, 3)
        .reshape(P, ne * kt, HIDDEN)
    ).astype(mnp)
    w3p = np.ascontiguousarray(
        W3[experts].reshape(ne, kt, P, 2 * out_half)
        .transpose(2, 0, 1, 3)
        .reshape(P, ne * kt, 2 * out_half)
    ).astype(mnp)
    b1p = np.ascontiguousarray(
        b1[experts].reshape(ne, kt, P).transpose(2, 0, 1).reshape(P, -1)
    )
    b2p = np.ascontiguousarray(
        b2[experts].reshape(ne, kt, P).transpose(2, 0, 1).reshape(P, -1)
    )
    b3p = np.ascontiguousarray(
        b3[experts].reshape(ne, 2, out_half).transpose(2, 0, 1)
    )

    in_maps = []
    for c in range(N_CORES):
        xc = x[gsafe[c]]
        xc[~valid[c]] = 0.0
        ec = eps[gsafe[c]]
        ec[~valid[c]] = 0.0
        xct = np.ascontiguousarray(xc.T)
        in_maps.append(
            {
                "xb": xct.astype(mnp) if mode != "f32" else xct,
                "st": np.ascontiguousarray(xct[:state_dim]),
                "epst": np.ascontiguousarray(ec.T),
                "w1": w1p,
                "w2": w2p,
                "w3": w3p,
                "b1": b1p,
                "b2": b2p,
                "b3": b3p,
            }
        )

    key = (tuple(slots), n_col, in_dim, state_dim, out_half, mode)
    nc = _get_nc(key, tuple(slots), n_col, in_dim, state_dim, out_half, mode)

    res = run_bass_kernel_spmd(nc, in_maps, list(range(N_CORES)), trace=trace)

    next_state = np.empty((B, state_dim), dtype=np.float32)
    reward = np.empty((B, 1), dtype=np.float32)
    for c in range(N_CORES):
        yt = res.results[c]["yt"]  # [out_half, n_col]
        cols = gidx[c][valid[c]]
        yv = yt[:, valid[c]]
        next_state[cols] = yv[:state_dim].T
        reward[cols, 0] = yv[state_dim]
    return (next_state, reward), res


def kernel(**inputs):
    out, _ = run(inputs)
    return out


# revision 4
# speedup vs baseline: 2.7960x; 1.0757x over previous
"""MoE-routed dynamics ensemble kernel for 8 Trainium2 NeuronCores.

Reference computes all 7 expert MLPs densely for every sample and then
gathers one expert per sample (idx in [0, TOP_K)).  Here we route instead:
sort samples by expert on the host, spread every expert's samples evenly
across the 8 cores, and run only the routed expert per sample as dense
per-expert matmuls in a feature-major layout ([features, samples]), which
needs no on-device transposes.  The Gaussian-sampling epilogue
(clip / exp / mu + std * eps, next_state = state + delta) runs on-chip.
"""

import sys

if "/opt/trn_rl_repo" not in sys.path:
    sys.path.insert(0, "/opt/trn_rl_repo")

import numpy as np

import concourse.bass as bass
import concourse.mybir as mybir
import concourse.tile as tile
import bass_rust
from concourse.bass_utils import run_bass_kernel_spmd

N_CORES = 8
HIDDEN = 512
P = 128
NT = 512          # max free dim per matmul (one PSUM bank of f32)
CB = 1024         # column block: matmul tiles sharing one weight load
F32 = mybir.dt.float32
BF16 = mybir.dt.bfloat16

# "f32" = exact-ish, "bf16" = bf16 matmuls (f32 accumulate + f32 epilogue)
MODE = "bf16"


def _split_multi_waits(nc):
    """This walrus build supports one semaphore wait per instruction; hoist
    extra waits onto NoOps placed just before the over-subscribed one."""
    counter = 0
    for f in nc.m.functions:
        for bb in f.blocks:
            new = []
            changed = False
            for inst in bb.instructions:
                si = inst.sync_info
                if si is not None and len(si.on_wait) > 1:
                    waits = list(si.on_wait)
                    for w in waits[:-1]:
                        counter += 1
                        nop = mybir.InstNoOp(
                            name=f"waitsplit-{counter}", ins=[], outs=[]
                        )
                        nop.engine = inst.engine
                        nop.sync_info = bass_rust.SyncInfo(
                            on_wait=[w], on_update=[]
                        )
                        new.append(nop)
                    inst.sync_info = bass_rust.SyncInfo(
                        on_wait=[waits[-1]], on_update=list(si.on_update)
                    )
                    changed = True
                new.append(inst)
            if changed:
                bb.instructions = new


def _build(slots, n_col, in_dim, state_dim, out_half, mode):
    """Build the SPMD Bass program.

    slots: list of capacities (columns) per expert slot, one slot per used
    expert, identical on every core.  n_col = sum(slots).
    """
    n_e = len(slots)
    kt = HIDDEN // P                     # contraction tiles for layers 2/3
    mt = HIDDEN // P                     # output row tiles for layers 1/2
    f32 = F32
    mdt = f32 if mode == "f32" else BF16  # matmul operand dtype
    relu = mybir.ActivationFunctionType.Relu
    expf = mybir.ActivationFunctionType.Exp
    ident = mybir.ActivationFunctionType.Identity
    alu = mybir.AluOpType

    out2 = 2 * out_half
    # per-expert weight blob columns: w1 | w2 (k-major) | w3 (k-major)
    W1C = HIDDEN
    W2C = kt * HIDDEN
    W3C = kt * out2
    WBC = W1C + W2C + W3C

    nc = bass.Bass("TRN2", debug=False)
    wb_d = nc.dram_tensor("wb", [n_e, P, WBC], mdt, kind="ExternalInput")
    bb_d = nc.dram_tensor("bb", [P, n_e, 2 * mt + 2], f32, kind="ExternalInput")
    xb_d = nc.dram_tensor("xb", [in_dim, n_col], mdt, kind="ExternalInput")
    st_d = nc.dram_tensor("st", [state_dim, n_col], f32, kind="ExternalInput")
    ep_d = nc.dram_tensor("epst", [out_half, n_col], f32, kind="ExternalInput")
    yt_d = nc.dram_tensor("yt", [out_half, n_col], f32, kind="ExternalOutput")

    with tile.TileContext(nc) as tc:
        with (
            tc.tile_pool(name="singles", bufs=1) as singles,
            tc.tile_pool(name="psum", bufs=4, space="PSUM") as psum,
            tc.tile_pool(name="acts", bufs=6) as acts,
            tc.tile_pool(name="epi", bufs=2) as epi,
        ):
            # DMA issue order = arrival priority: first expert's operands
            # first, later experts' weights stream in behind compute.
            wb_s = singles.tile([P, n_e, WBC], mdt, tag="wb")
            nc.sync.dma_start(out=wb_s[:, 0, :], in_=wb_d[0])
            xb_s = singles.tile([in_dim, n_col], mdt, tag="xb")
            nc.sync.dma_start(out=xb_s[:, 0 : slots[0]], in_=xb_d[:, 0 : slots[0]])
            bb_s = singles.tile([P, n_e, 2 * mt + 2], f32, tag="bb")
            nc.sync.dma_start(out=bb_s[:], in_=bb_d[:])
            if n_e > 1:
                nc.sync.dma_start(out=wb_s[:, 1, :], in_=wb_d[1])
            if n_col > slots[0]:
                nc.sync.dma_start(
                    out=xb_s[:, slots[0] :], in_=xb_d[:, slots[0] :]
                )
            ep_s = singles.tile([out_half, n_col], f32, tag="ep")
            nc.sync.dma_start(out=ep_s[:], in_=ep_d[:])
            st_s = singles.tile([state_dim, n_col], f32, tag="st")
            nc.sync.dma_start(out=st_s[:], in_=st_d[:])
            for s in range(2, n_e):
                nc.sync.dma_start(out=wb_s[:, s, :], in_=wb_d[s])

            def w1ap(s, m):
                return wb_s[0:in_dim, s, m * P : (m + 1) * P]

            def w2ap(s, k, m):
                c = W1C + k * HIDDEN + m * P
                return wb_s[:, s, c : c + P]

            def w3ap(s, k, half):
                c = W1C + W2C + k * out2 + half * out_half
                return wb_s[:, s, c : c + out_half]

            off = 0
            for s, cap in enumerate(slots):
                for cb0 in range(0, cap, CB):
                    cb = min(CB, cap - cb0)
                    c0 = off + cb0
                    # sub-tiles within the column block (share weight loads)
                    subs = [(o, min(NT, cb - o)) for o in range(0, cb, NT)]
                    # ---- layer 1: [in_dim -> HIDDEN] ----
                    a1 = []
                    for m in range(mt):
                        ps = psum.tile([P, cb], f32, tag="ps")
                        for o, n in subs:
                            nc.tensor.matmul(
                                ps[:, o : o + n],
                                w1ap(s, m),
                                xb_s[:, c0 + o : c0 + o + n],
                                start=True,
                                stop=True,
                            )
                        a = acts.tile([P, cb], mdt, tag="a1")
                        j = s * mt + m
                        nc.scalar.activation(
                            a, ps, relu, bias=bb_s[:, s, m : m + 1]
                        )
                        a1.append(a)
                    # ---- layer 2: [HIDDEN -> HIDDEN] ----
                    a2 = []
                    for m in range(mt):
                        ps = psum.tile([P, cb], f32, tag="ps")
                        for k in range(kt):
                            for o, n in subs:
                                nc.tensor.matmul(
                                    ps[:, o : o + n],
                                    w2ap(s, k, m),
                                    a1[k][:, o : o + n],
                                    start=(k == 0),
                                    stop=(k == kt - 1),
                                )
                        a = acts.tile([P, cb], mdt, tag="a2")
                        nc.vector.tensor_scalar(
                            a,
                            ps,
                            bb_s[:, s, mt + m : mt + m + 1],
                            0.0,
                            op0=alu.add,
                            op1=alu.max,
                        )
                        a2.append(a)
                    # ---- layer 3: [HIDDEN -> 2*out_half], mu | log_std ----
                    ps_mu = psum.tile([out_half, cb], f32, tag="ps")
                    for k in range(kt):
                        for o, n in subs:
                            nc.tensor.matmul(
                                ps_mu[:, o : o + n],
                                w3ap(s, k, 0),
                                a2[k][:, o : o + n],
                                start=(k == 0),
                                stop=(k == kt - 1),
                            )
                    ps_ls = psum.tile([out_half, cb], f32, tag="ps")
                    for k in range(kt):
                        for o, n in subs:
                            nc.tensor.matmul(
                                ps_ls[:, o : o + n],
                                w3ap(s, k, 1),
                                a2[k][:, o : o + n],
                                start=(k == 0),
                                stop=(k == kt - 1),
                            )
                    # ---- epilogue: y = mu + exp(min(ls, 2)) * eps ----
                    # (the reference also clips at -20; below that exp() is
                    # ~2e-9 so skipping the lower clip is far inside fp32
                    # noise for O(1) outputs)
                    t_ls = epi.tile([out_half, cb], f32, tag="ls")
                    nc.vector.tensor_scalar(
                        t_ls,
                        ps_ls,
                        bb_s[0:out_half, s, 2 * mt + 1 : 2 * mt + 2],
                        2.0,
                        op0=alu.add,
                        op1=alu.min,
                    )
                    t_std = epi.tile([out_half, cb], f32, tag="std")
                    nc.scalar.activation(t_std, t_ls, expf)
                    t_mu = epi.tile([out_half, cb], f32, tag="mu")
                    nc.scalar.activation(
                        t_mu, ps_mu, ident,
                        bias=bb_s[0:out_half, s, 2 * mt : 2 * mt + 1],
                    )
                    t_y = epi.tile([out_half, cb], f32, tag="y")
                    nc.vector.tensor_mul(t_y, t_std, ep_s[:, c0 : c0 + cb])
                    nc.vector.tensor_add(t_y, t_y, t_mu)
                    nc.vector.tensor_add(
                        t_y[0:state_dim],
                        t_y[0:state_dim],
                        st_s[:, c0 : c0 + cb],
                    )
                    nc.sync.dma_start(out=yt_d[:, c0 : c0 + cb], in_=t_y)
                off += cap

    _split_multi_waits(nc)
    return nc


_CACHE = {}


def _get_nc(key, *args):
    if key not in _CACHE:
        _CACHE[key] = _build(*args)
    return _CACHE[key]


def run(inputs, trace=False):
    state = np.asarray(inputs["state"], dtype=np.float32)
    action = np.asarray(inputs["action"], dtype=np.float32)
    eps = np.asarray(inputs["eps"], dtype=np.float32)
    idx = np.asarray(inputs["idx"]).astype(np.int64)
    W1 = np.asarray(inputs["W1"], dtype=np.float32)
    b1 = np.asarray(inputs["b1"], dtype=np.float32)
    W2 = np.asarray(inputs["W2"], dtype=np.float32)
    b2 = np.asarray(inputs["b2"], dtype=np.float32)
    W3 = np.asarray(inputs["W3"], dtype=np.float32)
    b3 = np.asarray(inputs["b3"], dtype=np.float32)

    B, state_dim = state.shape
    in_dim = state_dim + action.shape[1]
    out_half = state_dim + 1
    out2 = 2 * out_half
    n_ens = W1.shape[0]
    kt = HIDDEN // P
    mt = HIDDEN // P

    x = np.concatenate([state, action], axis=1)  # [B, in_dim]

    # ---- host routing: group samples by expert, balance across cores ----
    counts = np.bincount(idx, minlength=n_ens)
    experts = [e for e in range(n_ens) if counts[e] > 0]
    order = np.argsort(idx, kind="stable")
    seg_off = np.concatenate([[0], np.cumsum(counts)])

    slots = []
    for e in experts:
        cap = -(-int(counts[e]) // N_CORES)       # ceil
        cap = -(-cap // 4) * 4                     # mult of 4 cols (16B)
        slots.append(cap)
    n_col = sum(slots)

    # gather index per (core, column); -1 = padding
    gidx = np.full((N_CORES, n_col), -1, dtype=np.int64)
    off = 0
    for si, e in enumerate(experts):
        seg = order[seg_off[e] : seg_off[e + 1]]
        n = len(seg)
        base, rem = divmod(n, N_CORES)
        p = 0
        for c in range(N_CORES):
            ln = base + (1 if c < rem else 0)
            gidx[c, off : off + ln] = seg[p : p + ln]
            p += ln
        off += slots[si]

    valid = gidx >= 0
    gsafe = np.where(valid, gidx, 0)

    # ---- shared weight blobs ----
    mode = MODE
    if mode == "f32":
        mnp = np.float32
    else:
        import ml_dtypes

        mnp = ml_dtypes.bfloat16

    ne = len(experts)
    W1C = HIDDEN
    W2C = kt * HIDDEN
    WBC = W1C + W2C + kt * out2
    wb = np.zeros((ne, P, WBC), dtype=np.float32)
    for si, e in enumerate(experts):
        wb[si, :in_dim, :W1C] = W1[e]
        wb[si, :, W1C : W1C + W2C] = (
            W2[e].reshape(kt, P, HIDDEN).transpose(1, 0, 2).reshape(P, W2C)
        )
        wb[si, :, W1C + W2C :] = (
            W3[e].reshape(kt, P, out2).transpose(1, 0, 2).reshape(P, kt * out2)
        )
    wb = wb.astype(mnp)

    bbc = 2 * mt + 2
    bbl = np.zeros((P, ne, bbc), dtype=np.float32)
    for si, e in enumerate(experts):
        bbl[:, si, 0:mt] = b1[e].reshape(mt, P).T
        bbl[:, si, mt : 2 * mt] = b2[e].reshape(mt, P).T
        bbl[:out_half, si, 2 * mt] = b3[e][:out_half]
        bbl[:out_half, si, 2 * mt + 1] = b3[e][out_half:]

    in_maps = []
    for c in range(N_CORES):
        xc = x[gsafe[c]]
        xc[~valid[c]] = 0.0
        ec = eps[gsafe[c]]
        ec[~valid[c]] = 0.0
        xct = np.ascontiguousarray(xc.T)
        in_maps.append(
            {
                "wb": wb,
                "bb": bbl,
                "xb": xct.astype(mnp) if mode != "f32" else xct,
                "st": np.ascontiguousarray(xct[:state_dim]),
                "epst": np.ascontiguousarray(ec.T),
            }
        )

    key = (tuple(slots), n_col, in_dim, state_dim, out_half, mode)
    nc = _get_nc(key, tuple(slots), n_col, in_dim, state_dim, out_half, mode)

    res = run_bass_kernel_spmd(nc, in_maps, list(range(N_CORES)), trace=trace)

    next_state = np.empty((B, state_dim), dtype=np.float32)
    reward = np.empty((B, 1), dtype=np.float32)
    for c in range(N_CORES):
        yt = res.results[c]["yt"]  # [out_half, n_col]
        cols = gidx[c][valid[c]]
        yv = yt[:, valid[c]]
        next_state[cols] = yv[:state_dim].T
        reward[cols, 0] = yv[state_dim]
    return (next_state, reward), res


def kernel(**inputs):
    out, _ = run(inputs)
    return out
